# revision 18
# baseline (speedup 1.0000x reference)
"""Trainium2 Bass kernel for nn_MultiHeadAttention (B=2, S=2048, D=1024, H=16).

Sharding: 8 cores = 2 batches x 4 head-groups (4 heads / 256 dims each).
Each core computes its head-group's QKV projections, attention, and a
partial output projection (Megatron row-parallel); host sums the 4
partials per batch and adds the bias terms.

All operands are fp16. The Activation engine's 128 exp tiles (1038ns
each, 132.9us total) and the PE's 328.7k matmul cycles (~146us incl
decode) are the two rooflines; PE is the binding one, so the schedule
keeps PE busy continuously by interleaving projection/output work into
every attention section.

Section order is (h0,q0),(h1,q0),(h0,q1),(h1,q1),(h2,q0),(h3,q0),
(h2,q1),(h3,q1): heads 2,3 first touch state at section 4, so their
K/V/Q projections fill the mid sections, and the out-projection fills
sections 6-7. DMAs are issued from two queues (sync + gpsimd) because
a DMA holds its issuing sequencer for the whole transfer; inputs ride
sync, k-side prologue + transposes + outputs ride gpsimd.
"""
import sys
sys.path.insert(0, '/opt/trn_rl_repo')

from contextlib import ExitStack

import numpy as np

import concourse.bass as bass
import concourse.mybir as mybir
import concourse.tile as tile
from concourse import bacc
from concourse.bass_utils import run_bass_kernel_spmd

B, S, D, H = 2, 2048, 1024, 16
HD = D // H            # 64
NCORES = 8
GROUPS = 4             # head groups (tensor parallel)
DL = D // GROUPS       # 256 local d_out per core
HL = H // GROUPS       # 4 local heads
P = 128
KC = S // P            # 16 k-chunks
SC = D // P            # 8 d_in chunks
F16 = mybir.dt.float16
F32 = mybir.dt.float32


def _build_module():
    nc = bacc.Bacc(None, target_bir_lowering=False, debug=False)

    qT = nc.dram_tensor("qT", [D, S], F16, kind="ExternalInput").ap()
    kT = nc.dram_tensor("kT", [D, S], F16, kind="ExternalInput").ap()
    vT = nc.dram_tensor("vT", [D, S], F16, kind="ExternalInput").ap()
    wqT = nc.dram_tensor("wqT", [D, DL], F16, kind="ExternalInput").ap()
    wkT = nc.dram_tensor("wkT", [D, DL], F16, kind="ExternalInput").ap()
    wvT = nc.dram_tensor("wvT", [D, DL], F16, kind="ExternalInput").ap()
    woT = nc.dram_tensor("woT", [DL, D], F16, kind="ExternalInput").ap()
    bq2 = nc.dram_tensor("bq2", [2, P], F32, kind="ExternalInput").ap()
    bk2 = nc.dram_tensor("bk2", [2, P], F32, kind="ExternalInput").ap()
    idm = nc.dram_tensor("idm", [P, P], F16, kind="ExternalInput").ap()
    out = nc.dram_tensor("out", [S, D], F16, kind="ExternalOutput").ap()

    qTv = qT.rearrange("(kc p) s -> p kc s", p=P)
    kTv = kT.rearrange("(kc p) s -> p kc s", p=P)
    vTv = vT.rearrange("(kc p) s -> p kc s", p=P)
    wqv = wqT.rearrange("(kc p) m -> p kc m", p=P)
    wkv = wkT.rearrange("(kc p) m -> p kc m", p=P)
    wvv = wvT.rearrange("(kc p) m -> p kc m", p=P)
    outv = out.rearrange("(j p) n -> j p n", p=P)   # per-128-row blocks

    with tile.TileContext(nc) as tc:
        with ExitStack() as ctx:
            wpool = ctx.enter_context(tc.tile_pool(name="weights", bufs=1))
            big = ctx.enter_context(tc.tile_pool(name="big", bufs=1))
            qslab = ctx.enter_context(tc.tile_pool(name="qslab", bufs=4))
            kslab = ctx.enter_context(tc.tile_pool(name="kslab", bufs=4))
            vslab = ctx.enter_context(tc.tile_pool(name="vslab", bufs=4))
            ptp = ctx.enter_context(tc.tile_pool(name="pt", bufs=10))
            recp = ctx.enter_context(tc.tile_pool(name="rec", bufs=2))
            outp = ctx.enter_context(tc.tile_pool(name="outsb", bufs=4))

            # ---- persistent SBUF ----
            wq_sb = wpool.tile([P, SC, DL], F16)
            wk_sb = wpool.tile([P, SC, DL], F16)
            wv_sb = wpool.tile([P, SC, DL], F16)
            wo_sb = wpool.tile([P, DL // P, D], F16)
            bq_sb = wpool.tile([P, 2], F32)
            bk_sb = wpool.tile([P, 2], F32)
            QT = big.tile([P, 2, S], F16)           # [d_out in pair, m, q]
            KT = big.tile([P, 2, S], F16)
            V2 = big.tile([P, KC, HL, HD + 1], F16)  # [k, kc, head, V|one]
            xq0 = big.tile([P, KC, 2 * HD], F16)    # [q, qc, dv pair0]
            xq1 = big.tile([P, KC, 2 * HD], F16)
            xq = [xq0, xq1]
            xT = big.tile([P, 2, S], F16)           # [dv in pair, ci, q]

            nc.gpsimd.memset(V2[:, :, :, HD:HD + 1], 1.0)
            ident = wpool.tile([P, P], F16)
            nc.scalar.dma_start(ident[:], idm)

            # ---- PSUM: stA = score tiles (4 banks), stB = av+den (2),
            # stC = qk+pv (2, closed after section 5 for the out-proj
            # half-banks) ----
            stA = ExitStack()
            st_ps = stA.enter_context(
                tc.tile_pool(name="st_ps", bufs=2, space="PSUM"))   # 4 banks
            stB = ExitStack()
            av_ps = stB.enter_context(
                tc.tile_pool(name="av_ps", bufs=1, space="PSUM"))   # 2 banks
            stC = ExitStack()
            qk_ps = stC.enter_context(
                tc.tile_pool(name="qk_ps", bufs=1, space="PSUM"))   # 1 bank
            pv_ps = stC.enter_context(
                tc.tile_pool(name="pv_ps", bufs=1, space="PSUM"))   # 1 bank

            # ---- emission helpers (emission order == engine order) ----
            slabs = {}
            qk_pending = {}

            def load(kind, view, j, c0=0, c1=512, queue=None):
                pool = {"q": qslab, "k": kslab, "v": vslab}[kind]
                if (kind, j) in slabs:
                    t = slabs[(kind, j)]
                else:
                    t = pool.tile([P, SC, 512], F16, tag=kind, name="slab_t")
                    slabs[(kind, j)] = t
                eng = queue if queue is not None else nc.sync
                eng.dma_start(t[:, :, c0:c1],
                              view[:, :, j * 512 + c0:j * 512 + c1])

            def proj_qk(kind, j, m, lo=0, hi=SC, pool=None):
                """Matmuls [lo, hi) of the 8-chunk accumulation chain for
                the m-chunk of Q/K projection, slab j. Small parts keep a
                hook's PE steal under ~0.5us so exp never waits."""
                t = slabs[(kind, j)]
                w_sb, b_sb, dst = ((wq_sb, bq_sb, QT) if kind == "q"
                                   else (wk_sb, bk_sb, KT))
                if lo == 0:
                    pl = pool or qk_ps
                    ps = pl.tile([P, 512], F32,
                                 tag="qk" if pl is qk_ps else "st",
                                 name="ps")
                    qk_pending[(kind, j, m)] = ps
                else:
                    ps = qk_pending[(kind, j, m)]
                for kc in range(lo, hi):
                    nc.tensor.matmul(
                        ps[:], w_sb[:, kc, m * P:(m + 1) * P], t[:, kc, :],
                        start=(kc == 0), stop=(kc == SC - 1))
                if hi == SC:
                    del qk_pending[(kind, j, m)]
                    nc.vector.tensor_scalar_add(
                        dst[:, m, j * 512:(j + 1) * 512], ps[:],
                        b_sb[:, m:m + 1])

            def proj_v(c, h):
                """V2[:, c, h, :]: head h's V columns for k-chunk c."""
                t = slabs[("v", c // 4)]
                ss = c % 4
                psv = pv_ps.tile([P, HD], F32, tag="pv", name="psv")
                for kc in range(SC):
                    nc.tensor.matmul(
                        psv[:], t[:, kc, ss * P:(ss + 1) * P],
                        wv_sb[:, kc, h * HD:(h + 1) * HD],
                        start=(kc == 0), stop=(kc == SC - 1))
                nc.vector.tensor_copy(V2[:, c, h, 0:HD], psv[:])

            # ---- output projection: 2 rotating 1-bank PSUM tiles; one
            # osb tile + DMA per 128-row block j. copy_eng picks which
            # engine drains PSUM (DVE mid-stream; DVE/gpsimd at the tail,
            # with the DMA queue alternating sync/scalar). ----
            op_pool = [None]
            osbs = {}

            def op_tile():
                return op_pool[0].tile([P, 512], F32, tag="op", name="op")

            def osb_for(j):
                if j not in osbs:
                    osbs[j] = outp.tile([P, 1024], F16, tag="osb",
                                        name="osb")
                return osbs[j]

            def out_proj_full(j, n, copy_eng=None, queue=None):
                """n-half (512 cols) of output rows [128j, ..+128): both ci
                accumulated, copy to osb; DMA after the n=1 half."""
                op = op_tile()
                for ci in range(2):
                    nc.tensor.matmul(
                        op[:], xT[:, ci, j * P:(j + 1) * P],
                        wo_sb[:, ci, n * 512:(n + 1) * 512],
                        start=(ci == 0), stop=(ci == 1))
                osb = osb_for(j)
                if copy_eng == 'act':
                    nc.scalar.copy(osb[:, n * 512:(n + 1) * 512], op[:])
                else:
                    nc.vector.tensor_copy(
                        osb[:, n * 512:(n + 1) * 512], op[:])
                if n == 1:
                    (queue or nc.sync).dma_start(outv[j], osb[:])
                    del osbs[j]

            def transpose(pair, qc):
                nc.sync.dma_start_transpose(
                    xT[:, pair, qc * P:(qc + 1) * P], xq[pair][:, qc, :])

            def pe_transpose(pair, qc):
                """Tail transposes bypass the DMA queues: PE xbar via the
                identity, then a small DVE drain into xT. The fp16 result
                borrows an op-pool slot (256B of the 2KB bank)."""
                tp = op_pool[0].tile([P, P], F16, tag="op", name="tp")
                nc.tensor.matmul(tp[:], xq[pair][:, qc, :], ident[:],
                                 is_transpose=True, start=True, stop=True)
                nc.vector.tensor_copy(
                    xT[:, pair, qc * P:(qc + 1) * P], tp[:])

            def attention(h, qh, hooks, av_lag=1, carry_out=True,
                          post=None, sub=None):
                """One (head, q-half) pass; hooks[kc] runs before the kc's
                scores, post[kc] between the exp and the AV emission.
                av_lag: how many kc the AV matmuls trail scores/exp — a
                deeper lag rides out late-arriving V slabs without the
                in-order PE queue stalling the exp feed. carry_out=True
                returns the unemitted tail (last AVs + normalize) as
                closures for the caller to hook into the next section.
                sub=0/1 processes only the low/high 512 queries (one av
                bank, half-width score/exp tiles)."""
                hp, hm = (h % 2) * HD, h // 2
                pair = h // 2
                post = post or {}
                if sub is None:
                    # two av banks: [q, qc 0-3 | 4-7, V dims + denominator]
                    av_t = [av_ps.tile([P, 4, HD + 1], F32, tag="av_a",
                                       name="av_a"),
                            av_ps.tile([P, 4, HD + 1], F32, tag="av_b",
                                       name="av_b")]
                    qcs, qw = 8, 1024
                else:
                    av_t = [av_ps.tile([P, 4, HD + 1], F32,
                                       tag="av_a" if sub == 0 else "av_b",
                                       name="av_s")]
                    qcs, qw = 4, 512
                qbase = qh * 1024 + (sub or 0) * 512

                def emit_av(kc, pt):
                    # One start/stop per PSUM bank: start pends the whole
                    # 2KB zero region; later qc first-writes zero-fill it.
                    for qc in range(qcs):
                        nc.tensor.matmul(
                            av_t[qc // 4][:, qc % 4, :],
                            pt[:, qc * P:(qc + 1) * P],
                            V2[:, kc, h, :],
                            start=(kc == 0 and qc % 4 == 0),
                            stop=(kc == KC - 1 and qc % 4 == 3),
                            skip_group_check=True)

                def norm():
                    rec = recp.tile([P, 8], F32, tag="rec", name="rec")
                    qc0 = qh * 8 + (sub or 0) * 4
                    for i in range(len(av_t)):
                        nc.vector.reciprocal(
                            rec[:, i * 4:(i + 1) * 4], av_t[i][:, :, HD])
                        nc.vector.tensor_tensor(
                            xq[pair][:, qc0 + i * 4:qc0 + i * 4 + 4,
                                     hp:hp + HD],
                            av_t[i][:, :, 0:HD],
                            rec[:, i * 4:(i + 1) * 4, None].to_broadcast(
                                [P, 4, HD]),
                            mybir.AluOpType.mult)

                pending = []
                for kc in range(KC if carry_out else KC + av_lag):
                    for hook in hooks.get(kc, ()):
                        hook()
                    if kc < KC:
                        st = st_ps.tile([P, qw], F32, tag="st", name="st")
                        for qq in range(qw // 512):
                            q0 = qbase + qq * 512
                            nc.tensor.matmul(
                                st[:, qq * 512:(qq + 1) * 512],
                                KT[hp:hp + HD, hm, kc * P:(kc + 1) * P],
                                QT[hp:hp + HD, hm, q0:q0 + 512],
                                start=True, stop=True)
                        pt = ptp.tile([P, qw], F16, tag="pt", name="pt")
                        nc.scalar.activation(
                            pt[:], st[:],
                            mybir.ActivationFunctionType.Exp, scale=0.125)
                        pending.append((kc, pt))
                    for hook in post.get(kc, ()):
                        hook()
                    if len(pending) > (av_lag if kc < KC else 0):
                        emit_av(*pending.pop(0))
                if carry_out:
                    carry = [lambda kc=kc, pt=pt: emit_av(kc, pt)
                             for kc, pt in pending]
                    carry.append(norm)
                    return carry
                while pending:
                    emit_av(*pending.pop(0))
                norm()

            # ---- prologue ----
            # Two DMA queues: sync carries wq/q/v (+late weights), gpsimd
            # carries biases + wk/k0. A DMA holds its issuing sequencer for
            # the full transfer, so one queue would serialize everything.
            # tiny transfers ride the scalar queue; every big load rides
            # sync in consumption order (the DMA bus is serial anyway, and
            # sub-512B-descriptor loads pay a 2x latency multiplier, so
            # full-tile loads beat split ones)
            nc.scalar.dma_start(bq_sb[:], bq2.rearrange("m p -> p m"))
            nc.scalar.dma_start(bk_sb[:], bk2.rearrange("m p -> p m"))
            nc.sync.dma_start(wq_sb[:], wqv[:])
            load("q", qTv, 0)
            nc.sync.dma_start(wk_sb[:], wkv[:])
            load("k", kTv, 0)
            load("q", qTv, 1)
            load("k", kTv, 1)
            load("k", kTv, 2)
            nc.sync.dma_start(wv_sb[:], wvv[:])
            load("v", vTv, 0)
            load("k", kTv, 3)
            load("v", vTv, 1)
            load("v", vTv, 2)
            load("v", vTv, 3)
            load("q", qTv, 2)
            load("q", qTv, 3)
            nc.sync.dma_start(wo_sb[:], woT.rearrange("(c p) n -> p c n",
                                                      p=P))

            # back-to-back chains both ramp the PE p-state and do real
            # work; alternating psum banks avoids rotation stalls on the
            # bias-add drain
            proj_qk("q", 0, 0)
            proj_qk("q", 1, 0, pool=st_ps)
            proj_qk("k", 0, 0)

            def add_proj_hooks(hooks, kind, j, m, kc0):
                """Spread one projection chain over 4 hooks (2 matmuls
                each) at kc0..kc0+3 — each steals <0.5us of PE."""
                for ph in range(4):
                    hooks.setdefault(kc0 + ph, []).append(
                        lambda kind=kind, j=j, m=m, ph=ph:
                        proj_qk(kind, j, m, 2 * ph, 2 * ph + 2))

            def add_v_hooks(hooks, h, c_lo, c_hi, kc0):
                """proj_v for chunks [c_lo, c_hi) at hooks kc0, kc0+1, ..."""
                for i, c in enumerate(range(c_lo, c_hi)):
                    hooks.setdefault(kc0 + i, []).append(
                        lambda c=c, h=h: proj_v(c, h))

            # ---- S0 (h0, qh0): K m0 slabs just-in-time, V h0 as v
            # slabs land; deep av_lag rides out the v DMA latency. Every
            # section carries its AV tail + norm into the next section's
            # post-hooks so the exp stream never starves at boundaries. ----
            hooks = {2: [lambda: proj_qk("k", 1, 0, 0, 4)],
                     3: [lambda: proj_qk("k", 1, 0, 4, 8)],
                     7: [lambda: proj_qk("k", 2, 0, 0, 4)],
                     8: [lambda: proj_qk("k", 2, 0, 4, 8)],
                     11: [lambda: proj_qk("k", 3, 0, 0, 4)],
                     12: [lambda: proj_qk("k", 3, 0, 4, 8)]}
            add_v_hooks(hooks, 0, 0, 8, 8)   # pv(c) at kc c+8; AV at c+9
            ca = attention(0, 0, hooks, av_lag=9)   # AV c7-15 + norm

            # ---- S1 (h1, qh0) ----
            pv = lambda h, c: (lambda: proj_v(c, h))
            hooks = {}
            add_v_hooks(hooks, 1, 0, 11, 4)  # own V JIT: pv(c) at kc c+4
            add_proj_hooks(hooks, "q", 2, 0, 8)
            add_proj_hooks(hooks, "q", 3, 0, 12)
            post = {0: [pv(0, 8), pv(0, 9), ca[0], ca[1]],
                    1: [pv(0, 10), pv(0, 11), ca[2], ca[3]],
                    2: [pv(0, 12), pv(0, 13), ca[4], ca[5]],
                    3: [pv(0, 14), pv(0, 15), ca[6], ca[7]],
                    4: [ca[8], ca[9]]}
            ca = attention(1, 0, hooks, av_lag=5, post=post)  # c11-15+norm

            # ---- S2 (h0, qh1): transposes for pair0 qc0-7 once both
            # norms have run; Q m1 j0/j1 (for S4), K m1 j0 (for S4). ----
            hooks = {5: [lambda: transpose(0, 0), lambda: transpose(0, 1),
                         lambda: transpose(0, 2), lambda: transpose(0, 3)],
                     6: [lambda: transpose(0, 4), lambda: transpose(0, 5),
                         lambda: transpose(0, 6), lambda: transpose(0, 7)]}
            add_proj_hooks(hooks, "q", 0, 1, 1)
            add_proj_hooks(hooks, "q", 1, 1, 7)
            add_proj_hooks(hooks, "k", 0, 1, 11)
            post = {0: [pv(1, 11), pv(1, 12), ca[0]],
                    1: [pv(1, 13), pv(1, 14), ca[1], ca[2]],
                    2: [pv(1, 15), ca[3]],
                    3: [ca[4], ca[5]]}
            ca = attention(0, 1, hooks, av_lag=2, post=post)  # c14,15+norm

            # ---- S3 (h1, qh1): K m1 j1 + V h2 front half (for S4). ----
            hooks = {}
            add_proj_hooks(hooks, "k", 1, 1, 1)
            add_v_hooks(hooks, 2, 0, 8, 6)
            post = {0: [ca[0], ca[1]], 1: [ca[2]]}
            ca = attention(1, 1, hooks, av_lag=2, post=post)

            # ---- S4 (h2, qh0): pair0 qc8-15 transposes, K m1 j2/j3 JIT,
            # V h2 tail, Q m1 j2 (for S6). ----
            hooks = {3: [lambda: transpose(0, 8), lambda: transpose(0, 9),
                         lambda: transpose(0, 10),
                         lambda: transpose(0, 11)],
                     4: [lambda: transpose(0, 12), lambda: transpose(0, 13),
                         lambda: transpose(0, 14),
                         lambda: transpose(0, 15)]}
            add_v_hooks(hooks, 2, 8, 16, 0)
            add_proj_hooks(hooks, "k", 2, 1, 4)
            add_proj_hooks(hooks, "k", 3, 1, 8)
            add_proj_hooks(hooks, "q", 2, 1, 12)
            post = {0: [ca[0], ca[1]], 1: [ca[2]]}
            ca = attention(2, 0, hooks, av_lag=4, post=post)  # c12-15+norm

            # ---- S5 (h3, qh0): V h3 JIT + Q m1 j3 (for S6). ----
            hooks = {}
            add_v_hooks(hooks, 3, 0, 16, 0)
            add_proj_hooks(hooks, "q", 3, 1, 10)
            post = {0: [ca[0], ca[1]], 1: [ca[2], ca[3]], 2: [ca[4]]}
            ca = attention(3, 0, hooks, av_lag=4, post=post)

            # last qk/pv use was S5: free those banks for the out-proj
            stC.close()
            op_pool[0] = tc.alloc_tile_pool(name="op", bufs=2, space="PSUM",
                                            side="right")

            # ---- S6 (h2, qh1): pair1 qc0-7 transposes once S5's carried
            # norm lands, then out-proj j0-3. ----
            hooks = {3: [lambda: transpose(1, 0), lambda: transpose(1, 1),
                         lambda: transpose(1, 2), lambda: transpose(1, 3)],
                     4: [lambda: transpose(1, 4), lambda: transpose(1, 5),
                         lambda: transpose(1, 6), lambda: transpose(1, 7)]}
            for i, (j, n) in enumerate([(j, n) for j in range(4)
                                        for n in range(2)]):
                hooks.setdefault(i + 6, []).append(
                    lambda j=j, n=n: out_proj_full(j, n))
            post = {0: [ca[0], ca[1]], 1: [ca[2], ca[3]], 2: [ca[4]]}
            ca = attention(2, 1, hooks, av_lag=2, post=post)

            # ---- S7 (h3, qh1) in two 512-q subsections so half the
            # output tail runs inside the section's own window. ----
            hooks = {}
            for i, (j, n) in enumerate([(j, n) for j in range(4, 8)
                                        for n in range(2)]):
                hooks.setdefault(2 * i + 1, []).append(
                    lambda j=j, n=n: out_proj_full(j, n))
            post = {0: [ca[0], ca[1]], 1: [ca[2]]}
            ca = attention(3, 1, hooks, av_lag=2, post=post, sub=0)

            def tail_block(j, tp_qc=None):
                if tp_qc is not None:
                    pe_transpose(1, tp_qc)
                ce = 'act' if j % 2 else None
                qe = nc.scalar if j % 2 else nc.sync
                out_proj_full(j, 0, copy_eng=ce, queue=qe)
                out_proj_full(j, 1, copy_eng=ce, queue=qe)

            hooks = {1: [lambda: pe_transpose(1, 8),
                         lambda: pe_transpose(1, 9)],
                     2: [lambda: pe_transpose(1, 10)],
                     4: [lambda: tail_block(8)],
                     7: [lambda: tail_block(9, 11)],
                     10: [lambda: tail_block(10)],
                     13: [lambda: tail_block(11)]}
            post = {0: [ca[0], ca[1], ca[2]]}
            ca = attention(3, 1, hooks, av_lag=2, post=post, sub=1)

            # ---- tail: only qc12-15 remain ----
            for c in ca:
                c()
            pe_transpose(1, 12)
            pe_transpose(1, 13)
            for j in range(12, 14):
                pe_transpose(1, j + 2)
                ce = 'act' if j % 2 else None
                qe = nc.scalar if j % 2 else nc.sync
                out_proj_full(j, 0, copy_eng=ce, queue=qe)
                out_proj_full(j, 1, copy_eng=ce, queue=qe)
            # endgame: split each half across both copy engines and both
            # DMA queues so the final drain is as short as possible
            for j in (14, 15):
                for n in range(2):
                    op = op_tile()
                    for ci in range(2):
                        nc.tensor.matmul(
                            op[:], xT[:, ci, j * P:(j + 1) * P],
                            wo_sb[:, ci, n * 512:(n + 1) * 512],
                            start=(ci == 0), stop=(ci == 1))
                    osb = osb_for(j)
                    hs = osb[:, n * 512:(n + 1) * 512]
                    if n == 0:
                        nc.vector.tensor_copy(hs, op[:])
                        nc.sync.dma_start(outv[j][:, 0:512], hs)
                    else:
                        nc.scalar.copy(hs, op[:])
                        nc.scalar.dma_start(outv[j][:, 512:1024], hs)
                del osbs[j]
            op_pool[0].release()
            stB.close()
            stA.close()

    nc.compile()
    return nc


_NC = None


def _get_nc():
    global _NC
    if _NC is None:
        _NC = _build_module()
    return _NC


def kernel(query, key, value, mask, Wq, bq, Wk, bk, Wv, bv, Wo, bo,
           _trace=False):
    query = np.asarray(query, np.float32)
    key = np.asarray(key, np.float32)
    value = np.asarray(value, np.float32)
    Wq, Wk, Wv, Wo = (np.asarray(w, np.float32) for w in (Wq, Wk, Wv, Wo))
    bq, bk, bv, bo = (np.asarray(b_, np.float32) for b_ in (bq, bk, bv, bo))
    mask = np.asarray(mask, bool)

    f16 = lambda x: np.ascontiguousarray(x, np.float16)
    qT = [f16(query[b].T) for b in range(B)]
    kTh = [f16(key[b].T) for b in range(B)]
    vTh = [f16(value[b].T) for b in range(B)]

    in_maps = []
    for c in range(NCORES):
        b, g = c // GROUPS, c % GROUPS
        gs = slice(g * DL, (g + 1) * DL)
        in_maps.append({
            "qT": qT[b], "kT": kTh[b], "vT": vTh[b],
            "wqT": f16(Wq[gs, :].T),
            "wkT": f16(Wk[gs, :].T),
            "wvT": f16(Wv[gs, :].T),
            "woT": f16(Wo[:, gs].T),
            "bq2": np.ascontiguousarray(bq[gs].reshape(2, P)),
            "bk2": np.ascontiguousarray(bk[gs].reshape(2, P)),
            "idm": np.eye(P, dtype=np.float16),
        })

    nc = _get_nc()
    res = run_bass_kernel_spmd(nc, in_maps, core_ids=list(range(NCORES)),
                               trace=_trace)

    extra = (bv @ Wo.T + bo).astype(np.float32)  # bv folds through out-proj
    output = np.zeros((B, S, D), np.float32)
    for c in range(NCORES):
        output[c // GROUPS] += res.results[c]["out"].astype(np.float32)
    output += extra

    # masked query rows attend uniformly (softmax of constant -1e9)
    if mask.any():
        for b in range(B):
            rows = np.nonzero(mask[b, 0])[0]
            if rows.size:
                v_full = value[b] @ Wv.T + bv
                out_row = v_full.mean(0) @ Wo.T + bo
                output[b, rows, :] = out_row

    if _trace:
        return output, res
    return output
